# revision 40
# baseline (speedup 1.0000x reference)
"""Trainium2 Bass kernel for nn_DFMAtt: deformable-flow attention.

Per sample (1x1-conv proj, K=4 flow fields, softmax weights, bilinear
grid-sample of proj at flow-displaced positions, weighted sum over K).

Strategy (one batch sample per NeuronCore, 8 cores data-parallel):
  Flows are tiny (|f| < 2 px), so every bilinear corner lies in a fixed
  window dy,dx in [-2,2] around its output pixel.  The gather-and-blend
  becomes out = proj @ A with A banded (25 diagonals), built on-chip:
    - fused per-tile matmul emits proj (256) + flow/logit fields (12),
    - corner-weight planes M_s[n] on DVE in fp16 (k-outer layout,
      magic-number floor, fused scalar_tensor_tensor ops),
    - partition-shifted into source-index space via SBUF->SBUF DMAs in
      chunk-aligned groups (group i only needs planes of chunks <= i),
    - scattered into banded tiles A_r [128 x 516] with
      gpsimd.local_scatter, 3 r-tiles per call,
  and the main contraction runs on TensorE in fp16 (PSUM fp32
  accumulate; start=True zeroes the whole bank so partial-column pieces
  just accumulate).  All phases are chunked and pipelined so
  DMA/PE/DVE/Act/Pool overlap.
"""

import sys

sys.path.insert(0, "/opt/trn_rl_repo")

import numpy as np

import concourse.bass as bass
import concourse.mybir as mybir
from concourse import bacc
from concourse.bass import ts
from concourse.tile import TileContext

H = W = 96
C = 256
O = 256
K = 4
N = H * W            # 9216
NT = N // 128        # 72 position tiles
ALPHA = float(W) / float(W - 1)
DXS = list(range(-2, 3))   # -2..2
DYS = list(range(-2, 3))   # -2..2
SHIFTS = [(dy, dx) for dy in DYS for dx in DXS]
NS = len(SHIFTS)     # 25
NSP = 26             # padded to even for local_scatter
WOFF = 2 * W + 2     # 194; A_r covers n in [r*128 - WOFF, ...)
AW = 516             # window width; j = q + WOFF - delta_s in [0, 516)
NBLK = N // 512      # 18 output column blocks
TC = 18              # tiles per front chunk
NCH = NT // TC       # 4 chunks
RPT = 3              # r-tiles per scatter call
# shift sections (PE shift matmuls): section h covers dst tiles
# [HALVES[h], HALVES[h+1]); section h needs planes tiles < HALVES[h+1]+2
HALVES = [0, 34, 52, NT]

F32 = mybir.dt.float32
F16 = mybir.dt.float16
I16 = mybir.dt.int16
OP = mybir.AluOpType
AF = mybir.ActivationFunctionType

MAGIC = 12582912.0   # 1.5 * 2^23: fp32 round-to-int magic (ulp 1.0 zone)


def _host_consts(Wc, bc, Woff, boff, Wwt, bwt):
    """Host-side constant tensors baked into the NEFF."""
    # fused weight matrix [256, 268]: [Wc^T | a*Woff_x | a*Woff_y | Wwt^T]
    wf = np.concatenate(
        [
            Wc.T.astype(np.float32),                       # [c, 256]
            (ALPHA * Woff[:, 0, :]).T.astype(np.float32),  # [c, 4] fx_k
            (ALPHA * Woff[:, 1, :]).T.astype(np.float32),  # [c, 4] fy_k
            Wwt.T.astype(np.float32),                      # [c, 4]
        ],
        axis=1,
    ).astype(np.float16)

    # position fields: n = t*128 + p  ->  [p, t]
    n_grid = np.arange(N, dtype=np.int64).reshape(NT, 128).T   # [128, 72]
    gx = (n_grid % W).astype(np.float64)
    gy = (n_grid // W).astype(np.float64)

    # g' = (ix - gx) - 0.5 = a*fx + ((a-1)*gx + a*boff_x - 1.0), k-outer
    # [128, 4, 72].  The extra -0.5 turns the round-magic into floor:
    # floor(ix - gx) = round(g') = (g' + MAGIC) - MAGIC.
    gxc = ((ALPHA - 1.0) * gx[:, None, :]
           + (ALPHA * boff[:, 0].astype(np.float64))[None, :, None]
           - 1.0).astype(np.float16)
    gyc = ((ALPHA - 1.0) * gy[:, None, :]
           + (ALPHA * boff[:, 1].astype(np.float64))[None, :, None]
           - 1.0).astype(np.float16)
    # softmax bias as multiplicative factor exp(bwt_k): [128, 4]
    bwc = np.broadcast_to(np.exp(bwt.astype(np.float64))[None, :],
                          (128, K)).astype(np.float16).copy()
    # validity masks per shift column/row: [128, 5, 72]
    vxc = np.stack([((gx + dxv >= 0) & (gx + dxv <= W - 1)) for dxv in DXS],
                   axis=1).astype(np.float16)
    vyc = np.stack([((gy + dyv >= 0) & (gy + dyv <= H - 1)) for dyv in DYS],
                   axis=1).astype(np.float16)
    # conv bias row for per-tile bias matmul (even tiles) and broadcast
    # bias tile for the fused DVE psum copy (odd tiles)
    bias_row = np.concatenate(
        [bc.astype(np.float32), np.zeros(12, np.float32)])[None, :].astype(np.float16)
    bias_bcast = np.broadcast_to(bias_row, (128, 268)).copy()
    ones_row = np.ones((1, 128), dtype=np.float16)

    # scatter indices for a triple tile: j = q + WOFF - delta_s + 516*slot
    q = np.arange(128, dtype=np.int64)[:, None]
    deltas = np.array([dy * W + dx for dy, dx in SHIFTS], dtype=np.int64)[None, :]
    idx1 = (q + WOFF - deltas).astype(np.int16)            # [128, 25]
    assert idx1.min() >= 0 and idx1.max() < AW
    pad = np.full((128, 1), -1, dtype=np.int16)
    idxs = np.concatenate(
        [idx1, pad, idx1 + AW, pad, idx1 + 2 * AW, pad], axis=1)  # [128, 78]

    # partition-shift matrices for the planes m-shift on TensorE
    # (matmul out must start at partition 0, so each shift is two
    # full-partition matmuls with masked one-hot matrices):
    # s1_b[p, q] = 1 iff p == q - b (rows q >= b)
    # s2_b[p, q] = 1 iff p == q + 128 - b (rows q < b)
    qq = np.arange(128)
    scyc = {}
    for (dyv, dxv) in SHIFTS:
        b = (dyv * W + dxv) % 128
        if b not in scyc:
            m1 = np.zeros((128, 128), np.float16)
            m2 = np.zeros((128, 128), np.float16)
            sel = qq >= b
            m1[qq[sel] - b, qq[sel]] = 1.0
            m2[qq[~sel] + 128 - b, qq[~sel]] = 1.0
            scyc[b] = (m1, m2)
    return (wf, gxc, gyc, bwc, vxc, vyc, bias_row, bias_bcast, ones_row,
            idxs, scyc)


def build_program(Wc, bc, Woff, boff, Wwt, bwt):
    (wf_np, gxc_np, gyc_np, bwc_np, vxc_np, vyc_np, bias_np, biasb_np,
     ones_np, idxs_np, scyc_np) = _host_consts(Wc, bc, Woff, boff, Wwt, bwt)

    nc = bacc.Bacc()
    x_in = nc.dram_tensor("x", [C, N], F16, kind="ExternalInput")
    out_d = nc.dram_tensor("out", [O, N], F16, kind="ExternalOutput")

    wf_d = nc.inline_tensor(wf_np, "wf_c")
    gxc_d = nc.inline_tensor(gxc_np, "gxc_c")
    gyc_d = nc.inline_tensor(gyc_np, "gyc_c")
    bwc_d = nc.inline_tensor(bwc_np, "bwc_c")
    vxc_d = nc.inline_tensor(vxc_np, "vxc_c")
    vyc_d = nc.inline_tensor(vyc_np, "vyc_c")
    bias_d = nc.inline_tensor(bias_np, "bias_c")
    biasb_d = nc.inline_tensor(biasb_np, "biasb_c")
    ones_d = nc.inline_tensor(ones_np, "ones_c")
    idxs_d = nc.inline_tensor(idxs_np, "idxs_c")
    scyc_d = {
        b: (nc.inline_tensor(m1, f"scyc1_{b}"), nc.inline_tensor(m2, f"scyc2_{b}"))
        for b, (m1, m2) in scyc_np.items()
    }

    with TileContext(nc) as tc:
        with (
            tc.tile_pool(name="consts", bufs=1) as cpool,
            tc.tile_pool(name="big", bufs=1) as big,
            tc.tile_pool(name="wk", bufs=2) as wk,
            tc.tile_pool(name="apool", bufs=10) as apool,
            tc.tile_pool(name="obuf", bufs=4) as obuf,
            tc.tile_pool(name="ppsum", bufs=4, space="PSUM") as ppsum,
            tc.tile_pool(name="opsum", bufs=2, space="PSUM") as opsum,
            tc.tile_pool(name="spsum", bufs=1, space="PSUM") as spsum,
        ):
            # ---- constants into SBUF ----
            wf = cpool.tile([128, 2, 268], F16, tag="wf")
            nc.sync.dma_start(out=wf[:, 0], in_=wf_d[0:128, :])
            nc.sync.dma_start(out=wf[:, 1], in_=wf_d[128:256, :])
            gxc = cpool.tile([128, K, NT], F16, tag="gxc")
            nc.sync.dma_start(out=gxc[:], in_=gxc_d[:])
            gyc = cpool.tile([128, K, NT], F16, tag="gyc")
            nc.sync.dma_start(out=gyc[:], in_=gyc_d[:])
            bwc = cpool.tile([128, K], F16, tag="bwc")
            nc.sync.dma_start(out=bwc[:], in_=bwc_d[:])
            vxc = cpool.tile([128, 5, NT], F16, tag="vxc")
            nc.sync.dma_start(out=vxc[:], in_=vxc_d[:])
            vyc = cpool.tile([128, 5, NT], F16, tag="vyc")
            nc.sync.dma_start(out=vyc[:], in_=vyc_d[:])
            bias_sb = cpool.tile([1, 268], F16, tag="bias")
            nc.sync.dma_start(out=bias_sb[:], in_=bias_d[:])
            biasb_sb = cpool.tile([128, 268], F16, tag="biasb")
            nc.sync.dma_start(out=biasb_sb[:], in_=biasb_d[:])
            ones_sb = cpool.tile([1, 128], F16, tag="ones")
            nc.sync.dma_start(out=ones_sb[:], in_=ones_d[:])
            idxs_sb = cpool.tile([128, RPT * NSP], I16, tag="idxs")
            scyc = {}

            # ---- persistent big tiles ----
            xh = big.tile([128, 2, N], F16, tag="xh")
            # proj (cols 0:256) + fields (cols 256:268) per tile, fp16
            pt = big.tile([128, NT, 268], F16, tag="pt")
            # corner-weight planes in n-space / m-space, [s, t] layout
            planes = big.tile([128, NS, NT], F16, tag="planes")
            # scatter payload, [t, s] layout (padded s)
            mp = big.tile([128, NT, NSP], F16, tag="mp")

            nc.vector.memset(mp[:], 0.0)

            # ---- PE warm-up: keep TensorE busy during the x load so the
            # clock is at full p-state when the real matmuls arrive ----
            warm = ppsum.tile([128, 268], F32, tag="pp")
            for wi in range(24):
                nc.tensor.matmul(warm[:], wf[:, 0, 0:128], wf[:, 1],
                                 start=(wi == 0), stop=(wi == 23))
            wsink = wk.tile([128, 268], F16, tag="wsink")
            nc.scalar.activation(wsink[:], warm[:], AF.Copy)

            # ================= pipeline pieces =================
            def emit_chunk_front(c):
                """Fused matmuls + psum copies + DVE planes for chunk c."""
                t0, t1 = c * TC, (c + 1) * TC
                for t in range(t0, t1):
                    pp = ppsum.tile([128, 268], F32, tag="pp")
                    if t % 2 == 0:
                        # bias via K=1 matmul, psum copy on Act
                        nc.tensor.matmul(pp[:, 0:256], ones_sb[:],
                                         bias_sb[:, 0:256],
                                         start=True, stop=False)
                        nc.tensor.matmul(pp[:], xh[:, 0, ts(t, 128)], wf[:, 0],
                                         start=False, stop=False)
                        nc.tensor.matmul(pp[:], xh[:, 1, ts(t, 128)], wf[:, 1],
                                         start=False, stop=True)
                        nc.scalar.activation(pt[:, t, :], pp[:], AF.Copy)
                    else:
                        # bias fused into the DVE psum copy
                        nc.tensor.matmul(pp[:], xh[:, 0, ts(t, 128)], wf[:, 0],
                                         start=True, stop=False)
                        nc.tensor.matmul(pp[:], xh[:, 1, ts(t, 128)], wf[:, 1],
                                         start=False, stop=True)
                        nc.vector.scalar_tensor_tensor(
                            out=pt[:, t, :], in0=pp[:], scalar=1.0,
                            in1=biasb_sb[:], op0=OP.mult, op1=OP.add)

                # ---- DVE corner-weight pipeline (k-outer fp16) ----
                shp = [128, K, TC]

                def fields_ap(lo, hi):
                    # pt[:, t0:t1, lo:hi] iterated (k, t): strided fp16
                    return pt[:, t0:t1, lo:hi].transpose([0, 2, 1])

                def axis_pipe(fld_lo, gc, tag):
                    g = wk.tile(shp, F16, tag=f"g{tag}")
                    gf = wk.tile(shp, F16, tag=f"gf{tag}")
                    w1 = wk.tile(shp, F16, tag=f"w1{tag}")
                    w0 = wk.tile(shp, F16, tag=f"w0{tag}")
                    d0 = wk.tile(shp, F16, tag=f"d0{tag}")
                    nc.vector.tensor_tensor(out=g[:], in0=fields_ap(fld_lo, fld_lo + 4),
                                            in1=gc[:, :, t0:t1], op=OP.add)
                    # g holds (ix - gx) - 0.5; floor(ix-gx) = round(g) via magic
                    nc.vector.tensor_scalar(out=gf[:], in0=g[:],
                                            scalar1=MAGIC, scalar2=-MAGIC,
                                            op0=OP.add, op1=OP.add)
                    # wx1 = frac = (g + 0.5) - gf
                    nc.vector.scalar_tensor_tensor(out=w1[:], in0=g[:], scalar=0.5,
                                                   in1=gf[:], op0=OP.add,
                                                   op1=OP.subtract)
                    nc.vector.tensor_scalar(out=w0[:], in0=w1[:],
                                            scalar1=-1.0, scalar2=1.0,
                                            op0=OP.mult, op1=OP.add)
                    nc.vector.tensor_scalar(out=d0[:], in0=gf[:],
                                            scalar1=-2.0, scalar2=2.0,
                                            op0=OP.max, op1=OP.min)
                    return w0, w1, d0

                wx0, wx1, dx0 = axis_pipe(256, gxc, "x")
                wy0, wy1, dy0 = axis_pipe(260, gyc, "y")

                # softmax-normalized sampling weights e4n [128, K, TC]
                e4 = wk.tile(shp, F16, tag="e4")
                nc.scalar.activation(e4[:], fields_ap(264, 268), AF.Exp)
                e4b = wk.tile(shp, F16, tag="e4b")
                nc.vector.tensor_tensor(
                    out=e4b[:], in0=e4[:],
                    in1=bwc[:].unsqueeze(2).broadcast_to([128, K, TC]),
                    op=OP.mult)
                s01 = wk.tile([128, TC], F16, tag="s01")
                s23 = wk.tile([128, TC], F16, tag="s23")
                ssum = wk.tile([128, TC], F32, tag="ssum")
                rec = wk.tile([128, TC], F32, tag="rec")
                nc.vector.tensor_add(out=s01[:], in0=e4b[:, 0], in1=e4b[:, 1])
                nc.vector.tensor_add(out=s23[:], in0=e4b[:, 2], in1=e4b[:, 3])
                nc.vector.tensor_add(out=ssum[:], in0=s01[:], in1=s23[:])
                nc.vector.reciprocal(rec[:], ssum[:])
                e4n = wk.tile(shp, F16, tag="e4n")
                nc.vector.tensor_tensor(
                    out=e4n[:], in0=e4b[:],
                    in1=rec[:].unsqueeze(1).broadcast_to([128, K, TC]),
                    op=OP.mult)

                # horizontal / vertical corner-weight stacks [128, 5, K, TC]
                hx = wk.tile([128, 5, K, TC], F16, tag="hx")
                vy = wk.tile([128, 5, K, TC], F16, tag="vy")
                tmp = wk.tile(shp, F16, tag="tmp")
                for i, dxv in enumerate(DXS):
                    nc.vector.scalar_tensor_tensor(
                        out=hx[:, i], in0=dx0[:], scalar=float(dxv),
                        in1=wx0[:], op0=OP.is_equal, op1=OP.mult)
                    nc.vector.scalar_tensor_tensor(
                        out=tmp[:], in0=dx0[:], scalar=float(dxv - 1),
                        in1=wx1[:], op0=OP.is_equal, op1=OP.mult)
                    nc.vector.tensor_add(out=hx[:, i], in0=hx[:, i], in1=tmp[:])
                for i, dyv in enumerate(DYS):
                    nc.vector.scalar_tensor_tensor(
                        out=vy[:, i], in0=dy0[:], scalar=float(dyv),
                        in1=wy0[:], op0=OP.is_equal, op1=OP.mult)
                    nc.vector.scalar_tensor_tensor(
                        out=tmp[:], in0=dy0[:], scalar=float(dyv - 1),
                        in1=wy1[:], op0=OP.is_equal, op1=OP.mult)
                    nc.vector.tensor_add(out=vy[:, i], in0=vy[:, i], in1=tmp[:])
                # validity masks (+ e4n folded into vy)
                bshape = [128, 5, K, TC]
                vxa = vxc[:, :, t0:t1].unsqueeze(2).broadcast_to(bshape)
                vya = vyc[:, :, t0:t1].unsqueeze(2).broadcast_to(bshape)
                nc.vector.tensor_tensor(out=hx[:], in0=hx[:], in1=vxa, op=OP.mult)
                nc.vector.tensor_tensor(out=vy[:], in0=vy[:], in1=vya, op=OP.mult)
                nc.vector.tensor_tensor(
                    out=vy[:], in0=vy[:],
                    in1=e4n[:].unsqueeze(1).broadcast_to(bshape), op=OP.mult)

                # planes[s=(dy,dx), t] = sum_k vy[dy]*hx[dx]
                prod = wk.tile([128, 5, 5, K, TC], F16, tag="prod")
                for j in range(5):
                    nc.vector.tensor_tensor(
                        out=prod[:, j],
                        in0=vy[:, j].unsqueeze(1).broadcast_to(bshape),
                        in1=hx[:], op=OP.mult)
                q01 = wk.tile([128, 5, 5, TC], F16, tag="q01")
                q23 = wk.tile([128, 5, 5, TC], F16, tag="q23")
                nc.vector.tensor_tensor(out=q01[:], in0=prod[:, :, :, 0],
                                        in1=prod[:, :, :, 1], op=OP.add)
                nc.vector.tensor_tensor(out=q23[:], in0=prod[:, :, :, 2],
                                        in1=prod[:, :, :, 3], op=OP.add)
                pv = planes[:, :, t0:t1].rearrange("p (a b) t -> p a b t", a=5)
                nc.vector.tensor_tensor(out=pv, in0=q01[:], in1=q23[:], op=OP.add)

            def emit_shift_half(h):
                """m-shift of planes for dst tiles [t0,t1) via TensorE
                cyclic-shift matmuls, accumulated in 2 psum banks, then
                copied (transposed) into mp on DVE."""
                t0, t1 = HALVES[h], HALVES[h + 1]
                width = t1 - t0
                for sg, (s0, s1) in enumerate(((0, 13), (13, NS))):
                    nsg = s1 - s0
                    ps = spsum.tile([128, nsg, width], F32, tag=f"shps{sg}")
                    mms = []
                    for s in range(s0, s1):
                        dyv, dxv = SHIFTS[s]
                        delta = dyv * W + dxv
                        b = delta % 128
                        a = (delta - b) // 128
                        # piece 1 (rows q >= b): src tile t - a
                        p0, p1 = max(t0, a), min(t1, NT + a)
                        if p1 > p0:
                            mms.append((0, s, p0, p1, a))
                        # piece 2 (rows q < b): src tile t - a - 1
                        if b > 0:
                            p0, p1 = max(t0, a + 1), min(t1, NT + a + 1)
                            if p1 > p0:
                                mms.append((1, s, p0, p1, a + 1))
                    for i, (piece, s, p0, p1, aa) in enumerate(mms):
                        bb = (SHIFTS[s][0] * W + SHIFTS[s][1]) % 128
                        nc.tensor.matmul(
                            ps[:, s - s0, p0 - t0:p1 - t0],
                            scyc[bb][piece][:],
                            planes[:, s, p0 - aa:p1 - aa],
                            start=(i == 0),
                            stop=(i == len(mms) - 1),
                        )
                    # psum [128, nsg, width] -> mp[:, t0:t1, s0:s1] ([t, s])
                    nc.vector.tensor_copy(
                        out=mp[:, t0:t1, s0:s1],
                        in_=ps[:].transpose([0, 2, 1]))

            # scatter a triple of r-tiles into one banded buffer
            a_tiles = [None] * NT     # r -> (tile, col offset)

            def emit_scatter_triple(tau):
                r0 = tau * RPT
                nr = min(RPT, NT - r0)
                at = apool.tile([128, RPT * AW], F16, tag="a")
                nc.gpsimd.local_scatter(at[:, 0:nr * AW],
                                        mp[:, r0:r0 + nr, :],
                                        idxs_sb[:, 0:nr * NSP],
                                        channels=128, num_elems=nr * AW,
                                        num_idxs=nr * NSP)
                for i in range(nr):
                    a_tiles[r0 + i] = (at, i * AW)

            def emit_block(blk):
                n_lo, n_hi = blk * 512, blk * 512 + 512
                pieces = []
                for r in range(max(0, 4 * blk - 2), min(NT, 4 * blk + 6)):
                    w0 = r * 128 - WOFF
                    n0 = max(n_lo, w0)
                    n1 = min(n_hi, w0 + AW)
                    if n1 > n0:
                        pieces.append((r, n0, n1))
                for ohalf in range(2):
                    po = opsum.tile([128, 512], F32, tag="po")
                    for i, (r, n0, n1) in enumerate(pieces):
                        w0 = r * 128 - WOFF
                        at, off = a_tiles[r]
                        nc.tensor.matmul(
                            po[:, n0 - n_lo:n1 - n_lo],
                            pt[:, r, ts(ohalf, 128)],
                            at[:, off + n0 - w0:off + n1 - w0],
                            start=(i == 0),
                            stop=(i == len(pieces) - 1),
                        )
                    ob = obuf.tile([128, 512], F16, tag="ob")
                    if (blk + ohalf) % 3 == 0:
                        nc.vector.tensor_copy(out=ob[:], in_=po[:])
                    else:
                        nc.scalar.activation(ob[:], po[:], AF.Copy)
                    # trigger on the Act DGE queue: keeps the SP queue free
                    # for shift DMAs (no head-of-line blocking)
                    nc.scalar.dma_start(out=out_d[ts(ohalf, 128), n_lo:n_hi],
                                        in_=ob[:])

            # ================= emission schedule =================
            # x loads right after the light consts; the bulky scatter/shift
            # consts (1.3MB) load behind them off the critical path
            for c in range(NCH):
                sub = 3 if c == 0 else 1
                for j in range(sub):
                    t0 = c * TC + j * (TC // sub)
                    t1 = c * TC + (j + 1) * (TC // sub)
                    nc.sync.dma_start(out=xh[:, 0, t0 * 128:t1 * 128],
                                      in_=x_in[0:128, t0 * 128:t1 * 128])
                    nc.sync.dma_start(out=xh[:, 1, t0 * 128:t1 * 128],
                                      in_=x_in[128:256, t0 * 128:t1 * 128])
            nc.sync.dma_start(out=idxs_sb[:], in_=idxs_d[:])
            for b, (d1, d2) in scyc_d.items():
                t1_ = cpool.tile([128, 128], F16, tag=f"scyc1{b}")
                nc.sync.dma_start(out=t1_[:], in_=d1[:])
                t2_ = cpool.tile([128, 128], F16, tag=f"scyc2{b}")
                nc.sync.dma_start(out=t2_[:], in_=d2[:])
                scyc[b] = (t1_, t2_)

            # half 0's shift matmuls need planes of chunks 0-2 (src <= 35),
            # half 1 needs everything.  Scatters/blocks chase each half.
            next_blk = 0
            next_tau = 0

            def emit_scatters(done_tiles):
                nonlocal next_tau
                while next_tau * RPT + RPT <= done_tiles or (
                        done_tiles == NT and next_tau * RPT < NT):
                    emit_scatter_triple(next_tau)
                    next_tau += 1

            def emit_blocks():
                nonlocal next_blk
                r_hi = min(NT, next_tau * RPT)
                while (next_blk < NBLK
                       and min(NT, 4 * next_blk + 6) <= r_hi):
                    emit_block(next_blk)
                    next_blk += 1

            emit_chunk_front(0)
            emit_chunk_front(1)
            emit_chunk_front(2)
            emit_shift_half(0)
            emit_scatters(HALVES[1])
            emit_chunk_front(3)
            emit_blocks()
            emit_shift_half(1)
            emit_scatters(HALVES[2])
            emit_blocks()
            emit_shift_half(2)
            emit_scatters(NT)
            while next_blk < NBLK:
                emit_block(next_blk)
                next_blk += 1

    nc.finalize()
    return nc


_CACHE = {}


def _get_program(inputs):
    key = "prog"
    if key not in _CACHE:
        _CACHE[key] = build_program(
            np.asarray(inputs["Wc"], np.float32),
            np.asarray(inputs["bc"], np.float32),
            np.asarray(inputs["Woff"], np.float32),
            np.asarray(inputs["boff"], np.float32),
            np.asarray(inputs["Wwt"], np.float32),
            np.asarray(inputs["bwt"], np.float32),
        )
    return _CACHE[key]


def kernel(x, Wc, bc, Woff, boff, Wwt, bwt, _trace=False):
    from concourse.bass_utils import run_bass_kernel_spmd

    x = np.asarray(x, np.float32)
    b = x.shape[0]
    assert x.shape == (b, C, H, W) and b == 8

    nc = _get_program(dict(Wc=Wc, bc=bc, Woff=Woff, boff=boff, Wwt=Wwt, bwt=bwt))
    in_maps = [
        {"x": np.ascontiguousarray(x[i].reshape(C, N).astype(np.float16))}
        for i in range(b)
    ]
    res = run_bass_kernel_spmd(nc, in_maps, core_ids=list(range(b)), trace=_trace)
    _CACHE["last_results"] = res
    out = np.stack([res.results[i]["out"].reshape(O, H, W) for i in range(b)])
    return out.astype(np.float32)


# revision 42
# speedup vs baseline: 1.0374x; 1.0374x over previous
"""Trainium2 Bass kernel for nn_DFMAtt: deformable-flow attention.

Per sample (1x1-conv proj, K=4 flow fields, softmax weights, bilinear
grid-sample of proj at flow-displaced positions, weighted sum over K).

Strategy (one batch sample per NeuronCore, 8 cores data-parallel):
  Flows are tiny (|f| < 2 px), so every bilinear corner lies in a fixed
  window dy,dx in [-2,2] around its output pixel.  The gather-and-blend
  becomes out = proj @ A with A banded (25 diagonals), built on-chip:
    - fused per-tile matmul emits proj (256) + flow/logit fields (12),
    - corner-weight planes M_s[n] on DVE in fp16 (k-outer layout,
      magic-number floor, fused scalar_tensor_tensor ops),
    - partition-shifted into source-index space with TensorE one-hot
      shift matmuls in chunk-aligned sections (SBUF->SBUF DMAs would
      serialize the Sync sequencer at ~700ns per descriptor batch),
    - scattered into banded tiles A_r [128 x 516] with
      gpsimd.local_scatter, 3 r-tiles per call,
  and the main contraction runs on TensorE in fp16 (PSUM fp32
  accumulate; start=True zeroes the whole bank so partial-column pieces
  just accumulate).  All phases are chunked and pipelined so
  DMA/PE/DVE/Act/Pool overlap; out DMAs ride the Act DGE queue to keep
  the SP queue free for input loads.
"""

import sys

sys.path.insert(0, "/opt/trn_rl_repo")

import numpy as np

import concourse.bass as bass
import concourse.mybir as mybir
from concourse import bacc
from concourse.bass import ts
from concourse.tile import TileContext

H = W = 96
C = 256
O = 256
K = 4
N = H * W            # 9216
NT = N // 128        # 72 position tiles
ALPHA = float(W) / float(W - 1)
DXS = list(range(-2, 3))   # -2..2
DYS = list(range(-2, 3))   # -2..2
SHIFTS = [(dy, dx) for dy in DYS for dx in DXS]
NS = len(SHIFTS)     # 25
NSP = 26             # padded to even for local_scatter
WOFF = 2 * W + 2     # 194; A_r covers n in [r*128 - WOFF, ...)
AW = 516             # window width; j = q + WOFF - delta_s in [0, 516)
NBLK = N // 512      # 18 output column blocks
TC = 18              # tiles per front chunk
NCH = NT // TC       # 4 chunks
RPT = 3              # r-tiles per scatter call
# shift sections (PE shift matmuls): section h covers dst tiles
# [HALVES[h], HALVES[h+1]); section h needs planes tiles < HALVES[h+1]+2
HALVES = [0, 34, 52, 70, NT]

F32 = mybir.dt.float32
F16 = mybir.dt.float16
I16 = mybir.dt.int16
OP = mybir.AluOpType
AF = mybir.ActivationFunctionType

MAGIC = 12582912.0   # 1.5 * 2^23: fp32 round-to-int magic (ulp 1.0 zone)


def _host_consts(Wc, bc, Woff, boff, Wwt, bwt):
    """Host-side constant tensors baked into the NEFF."""
    # fused weight matrix [256, 268]: [Wc^T | a*Woff_x | a*Woff_y | Wwt^T]
    wf = np.concatenate(
        [
            Wc.T.astype(np.float32),                       # [c, 256]
            (ALPHA * Woff[:, 0, :]).T.astype(np.float32),  # [c, 4] fx_k
            (ALPHA * Woff[:, 1, :]).T.astype(np.float32),  # [c, 4] fy_k
            Wwt.T.astype(np.float32),                      # [c, 4]
        ],
        axis=1,
    ).astype(np.float16)

    # position fields: n = t*128 + p  ->  [p, t]
    n_grid = np.arange(N, dtype=np.int64).reshape(NT, 128).T   # [128, 72]
    gx = (n_grid % W).astype(np.float64)
    gy = (n_grid // W).astype(np.float64)

    # g' = (ix - gx) - 0.5 = a*fx + ((a-1)*gx + a*boff_x - 1.0), k-outer
    # [128, 4, 72].  The extra -0.5 turns the round-magic into floor:
    # floor(ix - gx) = round(g') = (g' + MAGIC) - MAGIC.
    gxc = ((ALPHA - 1.0) * gx[:, None, :]
           + (ALPHA * boff[:, 0].astype(np.float64))[None, :, None]
           - 1.0).astype(np.float16)
    gyc = ((ALPHA - 1.0) * gy[:, None, :]
           + (ALPHA * boff[:, 1].astype(np.float64))[None, :, None]
           - 1.0).astype(np.float16)
    # softmax bias as multiplicative factor exp(bwt_k): [128, 4]
    bwc = np.broadcast_to(np.exp(bwt.astype(np.float64))[None, :],
                          (128, K)).astype(np.float16).copy()
    # validity masks per shift column/row: [128, 5, 72]
    vxc = np.stack([((gx + dxv >= 0) & (gx + dxv <= W - 1)) for dxv in DXS],
                   axis=1).astype(np.float16)
    vyc = np.stack([((gy + dyv >= 0) & (gy + dyv <= H - 1)) for dyv in DYS],
                   axis=1).astype(np.float16)
    # conv bias row for per-tile bias matmul (even tiles) and broadcast
    # bias tile for the fused DVE psum copy (odd tiles)
    bias_row = np.concatenate(
        [bc.astype(np.float32), np.zeros(12, np.float32)])[None, :].astype(np.float16)
    bias_bcast = np.broadcast_to(bias_row, (128, 268)).copy()
    ones_row = np.ones((1, 128), dtype=np.float16)

    # scatter indices for a triple tile: j = q + WOFF - delta_s + 516*slot
    q = np.arange(128, dtype=np.int64)[:, None]
    deltas = np.array([dy * W + dx for dy, dx in SHIFTS], dtype=np.int64)[None, :]
    idx1 = (q + WOFF - deltas).astype(np.int16)            # [128, 25]
    assert idx1.min() >= 0 and idx1.max() < AW
    pad = np.full((128, 1), -1, dtype=np.int16)
    idxs = np.concatenate(
        [idx1, pad, idx1 + AW, pad, idx1 + 2 * AW, pad], axis=1)  # [128, 78]

    # partition-shift matrices for the planes m-shift on TensorE
    # (matmul out must start at partition 0, so each shift is two
    # full-partition matmuls with masked one-hot matrices):
    # s1_b[p, q] = 1 iff p == q - b (rows q >= b)
    # s2_b[p, q] = 1 iff p == q + 128 - b (rows q < b)
    qq = np.arange(128)
    scyc = {}
    for (dyv, dxv) in SHIFTS:
        b = (dyv * W + dxv) % 128
        if b not in scyc:
            m1 = np.zeros((128, 128), np.float16)
            m2 = np.zeros((128, 128), np.float16)
            sel = qq >= b
            m1[qq[sel] - b, qq[sel]] = 1.0
            m2[qq[~sel] + 128 - b, qq[~sel]] = 1.0
            scyc[b] = (m1, m2)
    return (wf, gxc, gyc, bwc, vxc, vyc, bias_row, bias_bcast, ones_row,
            idxs, scyc)


def build_program(Wc, bc, Woff, boff, Wwt, bwt):
    (wf_np, gxc_np, gyc_np, bwc_np, vxc_np, vyc_np, bias_np, biasb_np,
     ones_np, idxs_np, scyc_np) = _host_consts(Wc, bc, Woff, boff, Wwt, bwt)

    nc = bacc.Bacc()
    x_in = nc.dram_tensor("x", [C, N], F16, kind="ExternalInput")
    out_d = nc.dram_tensor("out", [O, N], F16, kind="ExternalOutput")

    wf_d = nc.inline_tensor(wf_np, "wf_c")
    gxc_d = nc.inline_tensor(gxc_np, "gxc_c")
    gyc_d = nc.inline_tensor(gyc_np, "gyc_c")
    bwc_d = nc.inline_tensor(bwc_np, "bwc_c")
    vxc_d = nc.inline_tensor(vxc_np, "vxc_c")
    vyc_d = nc.inline_tensor(vyc_np, "vyc_c")
    bias_d = nc.inline_tensor(bias_np, "bias_c")
    biasb_d = nc.inline_tensor(biasb_np, "biasb_c")
    ones_d = nc.inline_tensor(ones_np, "ones_c")
    idxs_d = nc.inline_tensor(idxs_np, "idxs_c")
    scyc_d = {
        b: (nc.inline_tensor(m1, f"scyc1_{b}"), nc.inline_tensor(m2, f"scyc2_{b}"))
        for b, (m1, m2) in scyc_np.items()
    }

    with TileContext(nc) as tc:
        with (
            tc.tile_pool(name="consts", bufs=1) as cpool,
            tc.tile_pool(name="big", bufs=1) as big,
            tc.tile_pool(name="wk", bufs=2) as wk,
            tc.tile_pool(name="apool", bufs=10) as apool,
            tc.tile_pool(name="obuf", bufs=4) as obuf,
            tc.tile_pool(name="ppsum", bufs=4, space="PSUM") as ppsum,
            tc.tile_pool(name="opsum", bufs=2, space="PSUM") as opsum,
            tc.tile_pool(name="spsum", bufs=1, space="PSUM") as spsum,
        ):
            # ---- constants into SBUF ----
            wf = cpool.tile([128, 2, 268], F16, tag="wf")
            nc.sync.dma_start(out=wf[:, 0], in_=wf_d[0:128, :])
            nc.sync.dma_start(out=wf[:, 1], in_=wf_d[128:256, :])
            gxc = cpool.tile([128, K, NT], F16, tag="gxc")
            nc.sync.dma_start(out=gxc[:], in_=gxc_d[:])
            gyc = cpool.tile([128, K, NT], F16, tag="gyc")
            nc.sync.dma_start(out=gyc[:], in_=gyc_d[:])
            bwc = cpool.tile([128, K], F16, tag="bwc")
            nc.sync.dma_start(out=bwc[:], in_=bwc_d[:])
            vxc = cpool.tile([128, 5, NT], F16, tag="vxc")
            nc.sync.dma_start(out=vxc[:], in_=vxc_d[:])
            vyc = cpool.tile([128, 5, NT], F16, tag="vyc")
            nc.sync.dma_start(out=vyc[:], in_=vyc_d[:])
            bias_sb = cpool.tile([1, 268], F16, tag="bias")
            nc.sync.dma_start(out=bias_sb[:], in_=bias_d[:])
            biasb_sb = cpool.tile([128, 268], F16, tag="biasb")
            nc.sync.dma_start(out=biasb_sb[:], in_=biasb_d[:])
            ones_sb = cpool.tile([1, 128], F16, tag="ones")
            nc.sync.dma_start(out=ones_sb[:], in_=ones_d[:])
            idxs_sb = cpool.tile([128, RPT * NSP], I16, tag="idxs")
            scyc = {}

            # ---- persistent big tiles ----
            xh = big.tile([128, 2, N], F16, tag="xh")
            # proj (cols 0:256) + fields (cols 256:268) per tile, fp16
            pt = big.tile([128, NT, 268], F16, tag="pt")
            # corner-weight planes in n-space / m-space, [s, t] layout
            planes = big.tile([128, NS, NT], F16, tag="planes")
            # scatter payload, [t, s] layout (padded s)
            mp = big.tile([128, NT, NSP], F16, tag="mp")

            nc.vector.memset(mp[:], 0.0)

            # ================= pipeline pieces =================
            def emit_chunk_front(c):
                """Fused matmuls + psum copies + DVE planes for chunk c."""
                t0, t1 = c * TC, (c + 1) * TC
                for t in range(t0, t1):
                    pp = ppsum.tile([128, 268], F32, tag="pp")
                    if t % 2 == 0:
                        # bias via K=1 matmul, psum copy on Act
                        nc.tensor.matmul(pp[:, 0:256], ones_sb[:],
                                         bias_sb[:, 0:256],
                                         start=True, stop=False)
                        nc.tensor.matmul(pp[:], xh[:, 0, ts(t, 128)], wf[:, 0],
                                         start=False, stop=False)
                        nc.tensor.matmul(pp[:], xh[:, 1, ts(t, 128)], wf[:, 1],
                                         start=False, stop=True)
                        nc.scalar.activation(pt[:, t, :], pp[:], AF.Copy)
                    else:
                        # bias fused into the DVE psum copy
                        nc.tensor.matmul(pp[:], xh[:, 0, ts(t, 128)], wf[:, 0],
                                         start=True, stop=False)
                        nc.tensor.matmul(pp[:], xh[:, 1, ts(t, 128)], wf[:, 1],
                                         start=False, stop=True)
                        nc.vector.scalar_tensor_tensor(
                            out=pt[:, t, :], in0=pp[:], scalar=1.0,
                            in1=biasb_sb[:], op0=OP.mult, op1=OP.add)

                # ---- DVE corner-weight pipeline (k-outer fp16) ----
                shp = [128, K, TC]

                def fields_ap(lo, hi):
                    # pt[:, t0:t1, lo:hi] iterated (k, t): strided fp16
                    return pt[:, t0:t1, lo:hi].transpose([0, 2, 1])

                def axis_pipe(fld_lo, gc, tag):
                    g = wk.tile(shp, F16, tag=f"g{tag}")
                    gf = wk.tile(shp, F16, tag=f"gf{tag}")
                    w1 = wk.tile(shp, F16, tag=f"w1{tag}")
                    w0 = wk.tile(shp, F16, tag=f"w0{tag}")
                    d0 = wk.tile(shp, F16, tag=f"d0{tag}")
                    nc.vector.tensor_tensor(out=g[:], in0=fields_ap(fld_lo, fld_lo + 4),
                                            in1=gc[:, :, t0:t1], op=OP.add)
                    # g holds (ix - gx) - 0.5; floor(ix-gx) = round(g) via magic
                    nc.vector.tensor_scalar(out=gf[:], in0=g[:],
                                            scalar1=MAGIC, scalar2=-MAGIC,
                                            op0=OP.add, op1=OP.add)
                    # wx1 = frac = (g + 0.5) - gf
                    nc.vector.scalar_tensor_tensor(out=w1[:], in0=g[:], scalar=0.5,
                                                   in1=gf[:], op0=OP.add,
                                                   op1=OP.subtract)
                    nc.vector.tensor_scalar(out=w0[:], in0=w1[:],
                                            scalar1=-1.0, scalar2=1.0,
                                            op0=OP.mult, op1=OP.add)
                    nc.vector.tensor_scalar(out=d0[:], in0=gf[:],
                                            scalar1=-2.0, scalar2=2.0,
                                            op0=OP.max, op1=OP.min)
                    return w0, w1, d0

                wx0, wx1, dx0 = axis_pipe(256, gxc, "x")
                wy0, wy1, dy0 = axis_pipe(260, gyc, "y")

                # softmax-normalized sampling weights e4n [128, K, TC]
                e4 = wk.tile(shp, F16, tag="e4")
                nc.scalar.activation(e4[:], fields_ap(264, 268), AF.Exp)
                e4b = wk.tile(shp, F16, tag="e4b")
                nc.vector.tensor_tensor(
                    out=e4b[:], in0=e4[:],
                    in1=bwc[:].unsqueeze(2).broadcast_to([128, K, TC]),
                    op=OP.mult)
                s01 = wk.tile([128, TC], F16, tag="s01")
                s23 = wk.tile([128, TC], F16, tag="s23")
                ssum = wk.tile([128, TC], F32, tag="ssum")
                rec = wk.tile([128, TC], F32, tag="rec")
                nc.vector.tensor_add(out=s01[:], in0=e4b[:, 0], in1=e4b[:, 1])
                nc.vector.tensor_add(out=s23[:], in0=e4b[:, 2], in1=e4b[:, 3])
                nc.vector.tensor_add(out=ssum[:], in0=s01[:], in1=s23[:])
                nc.vector.reciprocal(rec[:], ssum[:])
                e4n = wk.tile(shp, F16, tag="e4n")
                nc.vector.tensor_tensor(
                    out=e4n[:], in0=e4b[:],
                    in1=rec[:].unsqueeze(1).broadcast_to([128, K, TC]),
                    op=OP.mult)

                # horizontal / vertical corner-weight stacks [128, 5, K, TC]
                hx = wk.tile([128, 5, K, TC], F16, tag="hx")
                vy = wk.tile([128, 5, K, TC], F16, tag="vy")
                tmp = wk.tile(shp, F16, tag="tmp")
                for i, dxv in enumerate(DXS):
                    nc.vector.scalar_tensor_tensor(
                        out=hx[:, i], in0=dx0[:], scalar=float(dxv),
                        in1=wx0[:], op0=OP.is_equal, op1=OP.mult)
                    nc.vector.scalar_tensor_tensor(
                        out=tmp[:], in0=dx0[:], scalar=float(dxv - 1),
                        in1=wx1[:], op0=OP.is_equal, op1=OP.mult)
                    nc.vector.tensor_add(out=hx[:, i], in0=hx[:, i], in1=tmp[:])
                for i, dyv in enumerate(DYS):
                    nc.vector.scalar_tensor_tensor(
                        out=vy[:, i], in0=dy0[:], scalar=float(dyv),
                        in1=wy0[:], op0=OP.is_equal, op1=OP.mult)
                    nc.vector.scalar_tensor_tensor(
                        out=tmp[:], in0=dy0[:], scalar=float(dyv - 1),
                        in1=wy1[:], op0=OP.is_equal, op1=OP.mult)
                    nc.vector.tensor_add(out=vy[:, i], in0=vy[:, i], in1=tmp[:])
                # validity masks (+ e4n folded into vy)
                bshape = [128, 5, K, TC]
                vxa = vxc[:, :, t0:t1].unsqueeze(2).broadcast_to(bshape)
                vya = vyc[:, :, t0:t1].unsqueeze(2).broadcast_to(bshape)
                nc.vector.tensor_tensor(out=hx[:], in0=hx[:], in1=vxa, op=OP.mult)
                nc.vector.tensor_tensor(out=vy[:], in0=vy[:], in1=vya, op=OP.mult)
                nc.vector.tensor_tensor(
                    out=vy[:], in0=vy[:],
                    in1=e4n[:].unsqueeze(1).broadcast_to(bshape), op=OP.mult)

                # planes[s=(dy,dx), t] = sum_k vy[dy]*hx[dx]
                prod = wk.tile([128, 5, 5, K, TC], F16, tag="prod")
                for j in range(5):
                    nc.vector.tensor_tensor(
                        out=prod[:, j],
                        in0=vy[:, j].unsqueeze(1).broadcast_to(bshape),
                        in1=hx[:], op=OP.mult)
                q01 = wk.tile([128, 5, 5, TC], F16, tag="q01")
                q23 = wk.tile([128, 5, 5, TC], F16, tag="q23")
                nc.vector.tensor_tensor(out=q01[:], in0=prod[:, :, :, 0],
                                        in1=prod[:, :, :, 1], op=OP.add)
                nc.vector.tensor_tensor(out=q23[:], in0=prod[:, :, :, 2],
                                        in1=prod[:, :, :, 3], op=OP.add)
                pv = planes[:, :, t0:t1].rearrange("p (a b) t -> p a b t", a=5)
                nc.vector.tensor_tensor(out=pv, in0=q01[:], in1=q23[:], op=OP.add)

            def emit_shift_half(h):
                """m-shift of planes for dst tiles [t0,t1) via TensorE
                cyclic-shift matmuls, accumulated in 2 psum banks, then
                copied (transposed) into mp on DVE."""
                t0, t1 = HALVES[h], HALVES[h + 1]
                width = t1 - t0
                for sg, (s0, s1) in enumerate(((0, 13), (13, NS))):
                    nsg = s1 - s0
                    ps = spsum.tile([128, nsg, width], F32, tag=f"shps{sg}")
                    mms = []
                    for s in range(s0, s1):
                        dyv, dxv = SHIFTS[s]
                        delta = dyv * W + dxv
                        b = delta % 128
                        a = (delta - b) // 128
                        # piece 1 (rows q >= b): src tile t - a
                        p0, p1 = max(t0, a), min(t1, NT + a)
                        if p1 > p0:
                            mms.append((0, s, p0, p1, a))
                        # piece 2 (rows q < b): src tile t - a - 1
                        if b > 0:
                            p0, p1 = max(t0, a + 1), min(t1, NT + a + 1)
                            if p1 > p0:
                                mms.append((1, s, p0, p1, a + 1))
                    for i, (piece, s, p0, p1, aa) in enumerate(mms):
                        bb = (SHIFTS[s][0] * W + SHIFTS[s][1]) % 128
                        nc.tensor.matmul(
                            ps[:, s - s0, p0 - t0:p1 - t0],
                            scyc[bb][piece][:],
                            planes[:, s, p0 - aa:p1 - aa],
                            start=(i == 0),
                            stop=(i == len(mms) - 1),
                        )
                    # psum [128, nsg, width] -> mp[:, t0:t1, s0:s1] ([t, s])
                    nc.vector.tensor_copy(
                        out=mp[:, t0:t1, s0:s1],
                        in_=ps[:].transpose([0, 2, 1]))

            # scatter a triple of r-tiles into one banded buffer
            a_tiles = [None] * NT     # r -> (tile, col offset)

            def emit_scatter_triple(tau):
                r0 = tau * RPT
                nr = min(RPT, NT - r0)
                at = apool.tile([128, RPT * AW], F16, tag="a")
                nc.gpsimd.local_scatter(at[:, 0:nr * AW],
                                        mp[:, r0:r0 + nr, :],
                                        idxs_sb[:, 0:nr * NSP],
                                        channels=128, num_elems=nr * AW,
                                        num_idxs=nr * NSP)
                for i in range(nr):
                    a_tiles[r0 + i] = (at, i * AW)

            def emit_block(blk):
                n_lo, n_hi = blk * 512, blk * 512 + 512
                pieces = []
                for r in range(max(0, 4 * blk - 2), min(NT, 4 * blk + 6)):
                    w0 = r * 128 - WOFF
                    n0 = max(n_lo, w0)
                    n1 = min(n_hi, w0 + AW)
                    if n1 > n0:
                        pieces.append((r, n0, n1))
                for ohalf in range(2):
                    po = opsum.tile([128, 512], F32, tag="po")
                    for i, (r, n0, n1) in enumerate(pieces):
                        w0 = r * 128 - WOFF
                        at, off = a_tiles[r]
                        nc.tensor.matmul(
                            po[:, n0 - n_lo:n1 - n_lo],
                            pt[:, r, ts(ohalf, 128)],
                            at[:, off + n0 - w0:off + n1 - w0],
                            start=(i == 0),
                            stop=(i == len(pieces) - 1),
                        )
                    ob = obuf.tile([128, 512], F16, tag="ob")
                    nc.scalar.activation(ob[:], po[:], AF.Copy)
                    # trigger on the Act DGE queue: keeps the SP queue free
                    # for shift DMAs (no head-of-line blocking)
                    nc.scalar.dma_start(out=out_d[ts(ohalf, 128), n_lo:n_hi],
                                        in_=ob[:])

            # ================= emission schedule =================
            # x loads right after the light consts; the bulky scatter/shift
            # consts (1.3MB) load behind them off the critical path
            for c in range(NCH):
                sub = 3 if c == 0 else 1
                for j in range(sub):
                    t0 = c * TC + j * (TC // sub)
                    t1 = c * TC + (j + 1) * (TC // sub)
                    nc.sync.dma_start(out=xh[:, 0, t0 * 128:t1 * 128],
                                      in_=x_in[0:128, t0 * 128:t1 * 128])
                    nc.sync.dma_start(out=xh[:, 1, t0 * 128:t1 * 128],
                                      in_=x_in[128:256, t0 * 128:t1 * 128])
            nc.sync.dma_start(out=idxs_sb[:], in_=idxs_d[:])
            for b, (d1, d2) in scyc_d.items():
                t1_ = cpool.tile([128, 128], F16, tag=f"scyc1{b}")
                nc.sync.dma_start(out=t1_[:], in_=d1[:])
                t2_ = cpool.tile([128, 128], F16, tag=f"scyc2{b}")
                nc.sync.dma_start(out=t2_[:], in_=d2[:])
                scyc[b] = (t1_, t2_)

            # half 0's shift matmuls need planes of chunks 0-2 (src <= 35),
            # half 1 needs everything.  Scatters/blocks chase each half.
            next_blk = 0
            next_tau = 0

            def emit_scatters(done_tiles):
                nonlocal next_tau
                while next_tau * RPT + RPT <= done_tiles or (
                        done_tiles == NT and next_tau * RPT < NT):
                    emit_scatter_triple(next_tau)
                    next_tau += 1

            def emit_blocks():
                nonlocal next_blk
                r_hi = min(NT, next_tau * RPT)
                while (next_blk < NBLK
                       and min(NT, 4 * next_blk + 6) <= r_hi):
                    emit_block(next_blk)
                    next_blk += 1

            emit_chunk_front(0)
            emit_chunk_front(1)
            emit_chunk_front(2)
            emit_shift_half(0)
            emit_scatters(HALVES[1])
            emit_chunk_front(3)
            emit_blocks()
            emit_shift_half(1)
            emit_scatters(HALVES[2])
            emit_blocks()
            emit_shift_half(2)
            emit_scatters(HALVES[3])
            emit_blocks()
            emit_shift_half(3)
            emit_scatters(NT)
            while next_blk < NBLK:
                emit_block(next_blk)
                next_blk += 1

    nc.finalize()
    return nc


_CACHE = {}


def _get_program(inputs):
    key = "prog"
    if key not in _CACHE:
        _CACHE[key] = build_program(
            np.asarray(inputs["Wc"], np.float32),
            np.asarray(inputs["bc"], np.float32),
            np.asarray(inputs["Woff"], np.float32),
            np.asarray(inputs["boff"], np.float32),
            np.asarray(inputs["Wwt"], np.float32),
            np.asarray(inputs["bwt"], np.float32),
        )
    return _CACHE[key]


def kernel(x, Wc, bc, Woff, boff, Wwt, bwt, _trace=False):
    from concourse.bass_utils import run_bass_kernel_spmd

    x = np.asarray(x, np.float32)
    b = x.shape[0]
    assert x.shape == (b, C, H, W) and b == 8

    nc = _get_program(dict(Wc=Wc, bc=bc, Woff=Woff, boff=boff, Wwt=Wwt, bwt=bwt))
    in_maps = [
        {"x": np.ascontiguousarray(x[i].reshape(C, N).astype(np.float16))}
        for i in range(b)
    ]
    res = run_bass_kernel_spmd(nc, in_maps, core_ids=list(range(b)), trace=_trace)
    _CACHE["last_results"] = res
    out = np.stack([res.results[i]["out"].reshape(O, H, W) for i in range(b)])
    return out.astype(np.float32)
